# revision 7
# baseline (speedup 1.0000x reference)
"""3x3 SAME conv (224x224x128 -> 224x224x256) + ReLU on 8 TRN2 NeuronCores.

Spatial sharding over H: 28 output rows per core with a 1-row halo sliced
host-side (no collectives). Per core the conv is an implicit GEMM over the
width-padded (226-wide) row grid: for each 128-pixel output tile and each of
the 9 taps, one bf16 matmul contracts C_in=128 on the partition dim with the
tap's [128, 256] weight block as the moving operand, accumulating a
pixel-major [128 pix, 256 cout] PSUM tile; all 9 tap inputs are pure
flat-offset slices of a channel-major SBUF copy of the input (loaded with
xbar transpose DMAs, in 4 chunks so matmuls start early), and wraparounds at
row edges land on zero pad columns. Two PSUM accumulation groups are
interleaved on the PE, ReLU+bf16-cast drains on ACT/DVE, stores are
contiguous. A few junk matmuls on the first weight tile warm the PE clock
gate while the input loads. Host strips padding and upcasts to fp32.
"""

import sys

sys.path.insert(0, "/opt/trn_rl_repo")

import ml_dtypes
import numpy as np

H = 224
WID = 224
C_IN = 128
C_OUT = 256
KK = 3
NCORES = 8
RPC = H // NCORES
WP = WID + 2
HALO = RPC + 2
NPIX = HALO * WP
T_TILES = 50
YROWS = T_TILES * 128
XROWS = 6864
HALO_COLS = 2 * WP + 2 + 128

CH_T0 = [0, 13, 26, 39]
CH_TC = [13, 13, 13, 11]
CH_W = [((tc - 1) * 128 + HALO_COLS + 15) // 16 * 16 for tc in CH_TC]

N_WARM = 12

BF16 = ml_dtypes.bfloat16
COMPUTE_DT = ml_dtypes.bfloat16  # numpy-side compute dtype (fp16 measured 15% slower)

_COMPILED = None
LAST_RESULT = None


def _build(reps: int = 1):
    import concourse.bacc as bacc
    import concourse.mybir as mybir
    import concourse.tile as tile

    nc = bacc.Bacc("TRN2", target_bir_lowering=False, debug=False, num_devices=NCORES)
    MDT = mybir.dt.from_np(np.dtype(COMPUTE_DT))

    x_d = nc.dram_tensor("xs", [XROWS, C_IN], MDT, kind="ExternalInput").ap()
    w_d = nc.dram_tensor("w", [C_IN, 9 * C_OUT], MDT, kind="ExternalInput").ap()
    y_d = nc.dram_tensor("y", [YROWS, C_OUT], MDT, kind="ExternalOutput").ap()
    warm_d = nc.dram_tensor("warm", [128, 8], mybir.dt.float32, kind="ExternalOutput").ap()

    def body(tc, xt_pool, wts, out_pool, ps_pool, warm_pool):
        wps = ps_pool.tile([128, C_OUT], mybir.dt.float32, tag="ps")
        for i in range(N_WARM):
            nc.tensor.matmul(
                wps[:], wts[0][:, 0:128], wts[0][:],
                start=(i == 0), stop=(i == N_WARM - 1),
            )
        wsb = warm_pool.tile([128, 8], mybir.dt.float32)
        nc.any.tensor_copy(wsb[:], wps[:, 0:8])
        nc.sync.dma_start(warm_d[:], wsb[:])

        xts = []
        for c in range(4):
            xt = xt_pool.tile([C_IN, CH_W[c]], MDT, tag=f"xt{c}")
            base = CH_T0[c] * 128
            nc.sync.dma_start(xt[:], x_d[base : base + CH_W[c], :], transpose=True)
            xts.append(xt)

        def lhs(t, kh, kw):
            c = t // 13
            off = (t - CH_T0[c]) * 128 + kh * WP + kw
            return xts[c][:, off : off + 128]

        for tp in range(0, T_TILES, 2):
            ps0 = ps_pool.tile([128, C_OUT], mybir.dt.float32, tag="ps")
            ps1 = ps_pool.tile([128, C_OUT], mybir.dt.float32, tag="ps")
            for kh in range(KK):
                for kw in range(KK):
                    k = kh * KK + kw
                    nc.tensor.matmul(ps0[:], lhs(tp, kh, kw), wts[k][:],
                                     start=(k == 0), stop=(k == 8))
                    nc.tensor.matmul(ps1[:], lhs(tp + 1, kh, kw), wts[k][:],
                                     start=(k == 0), stop=(k == 8))
            for t, ps in ((tp, ps0), (tp + 1, ps1)):
                ot = out_pool.tile([128, C_OUT], MDT, tag="ot")
                nc.any.tensor_scalar_max(ot[:], ps[:], 0.0)
                nc.sync.dma_start(y_d[t * 128 : (t + 1) * 128, :], ot[:])

    with tile.TileContext(nc) as tc:
        with (
            tc.tile_pool(name="xt", bufs=(1 if reps == 1 else 2)) as xt_pool,
            tc.tile_pool(name="wt", bufs=1) as wt_pool,
            tc.tile_pool(name="out", bufs=6) as out_pool,
            tc.tile_pool(name="warm", bufs=1) as warm_pool,
            tc.tile_pool(name="ps", bufs=8, space="PSUM") as ps_pool,
        ):
            wts = []
            for k in range(9):
                wt = wt_pool.tile([C_IN, C_OUT], MDT, tag=f"w{k}")
                nc.sync.dma_start(wt[:], w_d[:, k * C_OUT : (k + 1) * C_OUT])
                wts.append(wt)
            if reps == 1:
                body(tc, xt_pool, wts, out_pool, ps_pool, warm_pool)
            else:
                with tc.For_i(0, reps, 1, hint_engines=(mybir.EngineType.PE,)):
                    body(tc, xt_pool, wts, out_pool, ps_pool, warm_pool)

    nc.compile()
    return nc


def _prep_inputs(x: np.ndarray, W: np.ndarray):
    xp = np.zeros((H + 2, WP, C_IN), np.float32)
    xp[1 : H + 1, 1 : WID + 1] = x
    xs = np.zeros((NCORES, XROWS, C_IN), COMPUTE_DT)
    for i in range(NCORES):
        xs[i, 1 : 1 + NPIX] = (
            xp[RPC * i : RPC * i + HALO].reshape(NPIX, C_IN).astype(COMPUTE_DT)
        )
    wh = (
        W.reshape(C_OUT, 9, C_IN)
        .transpose(2, 1, 0)
        .reshape(C_IN, 9 * C_OUT)
        .astype(COMPUTE_DT)
    )
    return xs, wh


def kernel(x: np.ndarray, W: np.ndarray) -> np.ndarray:
    global _COMPILED, LAST_RESULT
    from concourse import bass_utils

    if _COMPILED is None:
        _COMPILED = _build()
    nc = _COMPILED

    xs, wh = _prep_inputs(np.asarray(x, np.float32), np.asarray(W, np.float32))
    in_maps = [{"xs": np.ascontiguousarray(xs[i]), "w": wh} for i in range(NCORES)]

    try:
        res = bass_utils.run_bass_kernel_spmd(nc, in_maps, core_ids=list(range(NCORES)))
    except Exception:
        import os

        if os.environ.get("BASS_TRACE"):
            os.environ.pop("BASS_TRACE", None)
            res = bass_utils.run_bass_kernel_spmd(
                nc, in_maps, core_ids=list(range(NCORES))
            )
        else:
            raise
    LAST_RESULT = res

    y = np.stack([r["y"] for r in res.results])
    y = y[:, : RPC * WP].reshape(NCORES, RPC, WP, C_OUT)[:, :, 1 : WID + 1]
    return y.reshape(H, WID, C_OUT).astype(np.float32)
